# revision 9
# baseline (speedup 1.0000x reference)
"""MultiHeadAttention Bass kernel for TRN2, 8-core SPMD.

Sharding: core c -> batch b = c//4, heads [4*(c%4), 4*(c%4)+4).
Each core computes the qkv projection for its 4 heads, rope, attention,
and the out-projection partial (its 256 channels); host sums partials.

Device tensors (per core, host-prepped):
  xT        [1024, 2048]  bf16   x[b].T (channels on partitions)
  wqkT      [1024, 512]   bf16   q/k weight cols perm: [qA(128)|qB(128)|kA(128)|kB(128)]
                                 qA = even d-idx of 4 heads (4x32), qB = odds
  qkb       [128, 4]      f32    per-partition bias for the 4 o-tiles
  wvT       [1024, 256]   bf16   v weights, natural order
  vb_bc     [128, 256]    f32    v bias broadcast across partitions
  cos4/sin4 [128, 2048]   bf16   rope tables tiled 4x heads
  ind       [128, 128]    f32    row 64, cols 0:64 = 1 (recip broadcast matmul)
  projwT    [256, 1024]   bf16   out_w cols for this core's heads, transposed
  y         [1024, 2048]  f32    OUT: partial y^T (m on rows)

v2: scores via fp8e4 DoubleRow matmuls (K=64 even/odd packed as 2
k-subtiles of 32) -> half the score matmuls at 2x rate.  Rope outputs
written by DVE directly as fp8 into [128, 2, T] tiles.  QKV bias adds
on ACT (idle in phase A), normalization mul + y PSUM->SBUF copies on
Pool/GpSimd, un written in-place into the packed projection tile.
"""

import numpy as np
import ml_dtypes

import concourse.bass as bass
import concourse.tile as tile
from concourse import bacc, mybir
from concourse.bass import ts

F32 = mybir.dt.float32
BF16 = mybir.dt.bfloat16
FP8 = mybir.dt.float8e4
AF = mybir.ActivationFunctionType
DR = mybir.MatmulPerfMode.DoubleRow

B, T, DIM, NH = 2, 2048, 1024, 16
HD = 64          # head dim
HPC = 4          # heads per core
TC = 512         # t-chunk (one psum bank / fp32 matmul N limit)
TH = 1024        # t-half (exp op size)
NST = T // 128   # 16 s-tiles


def build(n_cores=8, loop_reps=1):
    nc = bacc.Bacc("TRN2", target_bir_lowering=False, debug=False,
                   num_devices=n_cores)

    xT_d = nc.dram_tensor("xT", [DIM, T], BF16, kind="ExternalInput").ap()
    wqkT_d = nc.dram_tensor("wqkT", [DIM, 512], BF16, kind="ExternalInput").ap()
    qkb_d = nc.dram_tensor("qkb", [128, 4], F32, kind="ExternalInput").ap()
    wvT_d = nc.dram_tensor("wvT", [DIM, 256], BF16, kind="ExternalInput").ap()
    vbbc_d = nc.dram_tensor("vb_bc", [128, 256], F32, kind="ExternalInput").ap()
    cos4_d = nc.dram_tensor("cos4", [128, T], BF16, kind="ExternalInput").ap()
    sin4_d = nc.dram_tensor("sin4", [128, T], BF16, kind="ExternalInput").ap()
    ind_d = nc.dram_tensor("ind", [128, 128], F32, kind="ExternalInput").ap()
    projwT_d = nc.dram_tensor("projwT", [256, 1024], BF16, kind="ExternalInput").ap()
    y_d = nc.dram_tensor("y", [DIM, T], F32, kind="ExternalOutput").ap()

    with tile.TileContext(nc) as tc:
        if loop_reps > 1:
            with tc.For_i(0, loop_reps, 1):
                _kernel(nc, tc, xT_d, wqkT_d, qkb_d, wvT_d, vbbc_d, cos4_d,
                        sin4_d, ind_d, projwT_d, y_d)
        else:
            _kernel(nc, tc, xT_d, wqkT_d, qkb_d, wvT_d, vbbc_d, cos4_d,
                    sin4_d, ind_d, projwT_d, y_d)
    nc.compile()
    return nc


def _kernel(nc, tc, xT_d, wqkT_d, qkb_d, wvT_d, vbbc_d, cos4_d, sin4_d,
            ind_d, projwT_d, y_d):
    from contextlib import ExitStack
    ctx = ExitStack()
    with ctx:
        # ---- constant / weight pools ----
        consts = ctx.enter_context(tc.tile_pool(name="consts", bufs=1))
        xpool = ctx.enter_context(tc.tile_pool(name="xp", bufs=1))

        # split input loads across both HWDGE queues (SP + ACT): critical
        # path (xT, wqk) on SP, the rest on the scalar queue
        xT = [xpool.tile([128, T], BF16, tag=f"xT{j}", name=f"xT{j}") for j in range(8)]
        for j in range(8):
            nc.sync.dma_start(xT[j][:], xT_d[ts(j, 128), :])
        wqk = [consts.tile([128, 512], BF16, tag=f"wqk{j}", name=f"wqk{j}") for j in range(8)]
        for j in range(8):
            nc.sync.dma_start(wqk[j][:], wqkT_d[ts(j, 128), :])
        wv = [consts.tile([128, 256], BF16, tag=f"wv{j}", name=f"wv{j}") for j in range(8)]
        for j in range(8):
            nc.scalar.dma_start(wv[j][:], wvT_d[ts(j, 128), :])
        qkb = consts.tile([128, 4], F32, tag="qkb")
        nc.sync.dma_start(qkb[:], qkb_d[:])
        vbbc = consts.tile([128, 256], F32, tag="vbbc")
        nc.scalar.dma_start(vbbc[:], vbbc_d[:])
        cos4 = consts.tile([128, T], BF16, tag="cos4")
        nc.scalar.dma_start(cos4[:], cos4_d[:])
        sin4 = consts.tile([128, T], BF16, tag="sin4")
        nc.scalar.dma_start(sin4[:], sin4_d[:])
        ind = consts.tile([128, 128], F32, tag="ind")
        nc.scalar.dma_start(ind[:], ind_d[:])
        projw = [consts.tile([128, 1024], BF16, tag=f"pw{g}", name=f"pw{g}") for g in range(2)]
        for g in range(2):
            nc.scalar.dma_start(projw[g][:], projwT_d[ts(g, 128), :])

        # ---- single shared PSUM pool: 4 tags x 2 banks = 8 banks ----
        ps = ctx.enter_context(tc.tile_pool(name="ps", bufs=1, space="PSUM"))

        def ps_tile(shape, tag):
            return ps.tile(shape, F32, tag=tag, name=f"ps_{tag}")

        # ---- phase A: QKV projection ----
        qksb = ctx.enter_context(tc.tile_pool(name="qksb", bufs=1))
        # o-tiles: 0=qA(evens) 1=qB(odds) 2=kA 3=kB
        qkt = [qksb.tile([128, T], BF16, tag=f"qk{o}", name=f"qk{o}") for o in range(4)]
        stags = ("sA", "sB")
        utags = ("uA", "uB")
        # phase A accumulates in the u-banks so that in the repeat loop the
        # next iteration's qkv overlaps this iteration's projection (s-banks)
        for o in range(4):
            for c in range(4):
                pst = ps_tile([128, TC], utags[(o * 4 + c) % 2])
                for j in range(8):
                    nc.tensor.matmul(pst[:], wqk[j][:, ts(o, 128)],
                                     xT[j][:, ts(c, TC)],
                                     start=(j == 0), stop=(j == 7))
                # bias add on ACT (idle during phase A); DVE is busy w/ rope
                nc.scalar.activation(qkt[o][:, ts(c, TC)], pst[:],
                                     AF.Identity, bias=qkb[:, o:o + 1])

        # v projection: [t, d'] layout, + ones column per head block
        vsb = ctx.enter_context(tc.tile_pool(name="vsb", bufs=1))
        vt = [vsb.tile([128, 260], BF16, tag=f"v{i}", name=f"v{i}") for i in range(NST)]
        for i in range(NST):
            pst = ps_tile([128, 256], utags[i % 2])
            for j in range(8):
                nc.tensor.matmul(pst[:], xT[j][:, ts(i, 128)], wv[j][:],
                                 start=(j == 0), stop=(j == 7))
            nc.vector.memset(vt[i][:], 1.0)
            # v block h at cols 65h:65h+64; col 65h+64 stays 1.0
            nc.vector.tensor_add(
                vt[i][:].rearrange("p (h d) -> p h d", h=4)[:, :, 0:64],
                pst[:].rearrange("p (h d) -> p h d", h=4),
                vbbc[:].rearrange("p (h d) -> p h d", h=4))

        # ---- rope: fp8 outputs in DoubleRow k-subtile layout ----
        # qEO/kEO [128, 2, T] fp8: [:,0,:] = rotated evens, [:,1,:] = odds
        ropet = ctx.enter_context(tc.tile_pool(name="ropet", bufs=2))
        rotsb = ctx.enter_context(tc.tile_pool(name="rotsb", bufs=1))
        rot = {}
        for src in ("q", "k"):
            a = qkt[0] if src == "q" else qkt[2]   # evens
            b = qkt[1] if src == "q" else qkt[3]   # odds
            eo = rotsb.tile([128, 2, T], FP8, tag=f"{src}EO", name=f"{src}EO")
            t1 = ropet.tile([128, T], BF16, tag="t1")
            t2 = ropet.tile([128, T], BF16, tag="t2")
            nc.vector.tensor_mul(t1[:], a[:], cos4[:])
            nc.vector.tensor_mul(t2[:], b[:], sin4[:])
            nc.vector.tensor_sub(eo[:, 0, :], t1[:], t2[:])
            t3 = ropet.tile([128, T], BF16, tag="t3")
            t4 = ropet.tile([128, T], BF16, tag="t4")
            nc.vector.tensor_mul(t3[:], a[:], sin4[:])
            nc.vector.tensor_mul(t4[:], b[:], cos4[:])
            nc.vector.tensor_add(eo[:, 1, :], t3[:], t4[:])
            rot[src] = eo
        qEO = rot["q"]
        kEO = rot["k"]

        # ---- attention passes: (pair g, t-half th) ----
        ppool = ctx.enter_context(tc.tile_pool(name="pp", bufs=2))
        npool = ctx.enter_context(tc.tile_pool(name="np", bufs=2))
        unsb = ctx.enter_context(tc.tile_pool(name="unsb", bufs=1))
        ysb = ctx.enter_context(tc.tile_pool(name="ysb", bufs=3))

        # packed u_norm for the projection: [128 (pair c'), TH] per (g, th)
        upk = [[unsb.tile([128, TH], BF16, tag=f"upk{g}{th}", name=f"upk{g}{th}") for th in range(2)]
               for g in range(2)]

        for g in range(2):
            for th in range(2):
                t0 = th * TH
                hA, hB = 2 * g, 2 * g + 1
                u = {h: ps_tile([65, TH], utags[h - 2 * g]) for h in (hA, hB)}
                for i in range(NST):
                    # scores S^T [s-tile, t-half], one fp8 DoubleRow matmul
                    # per (head, chunk): K=64 (2 k-subtiles of 32 e/o rows)
                    sps = {h: ps_tile([128, TH], stags[h - 2 * g])
                           for h in (hA, hB)}
                    for c in range(2):
                        for h in (hA, hB):
                            r = slice(32 * h, 32 * h + 32)
                            nc.tensor.matmul(
                                sps[h][:, ts(c, TC)],
                                kEO[r, :, ts(i, 128)],
                                qEO[r, :, t0 + c * TC:t0 + (c + 1) * TC],
                                start=True, stop=True, perf_mode=DR,
                                tile_position=(32 * h, 0))
                    for h in (hA, hB):
                        p = ppool.tile([128, TH], BF16, tag=f"p{h - 2 * g}", name=f"p{h - 2 * g}")
                        nc.scalar.activation(p[:], sps[h][:], AF.Exp,
                                             scale=0.125)
                        # AV + den: lhsT = [v_h | 1] (65 cols)
                        for c in range(2):
                            nc.tensor.matmul(
                                u[h][:, ts(c, TC)],
                                vt[i][:, 65 * h:65 * h + 65],
                                p[:, ts(c, TC)],
                                start=(i == 0), stop=(i == NST - 1))
                # normalization for this pass
                for h in (hA, hB):
                    dinv = npool.tile([65, TH], F32, tag="dinv", name="dinv")
                    nc.vector.reciprocal(dinv[64:65, :], u[h][64:65, :])
                    bc = ps_tile([128, TH], stags[h - 2 * g])
                    for c in range(2):
                        nc.tensor.matmul(bc[:, ts(c, TC)],
                                         ind[64:65, :], dinv[64:65, ts(c, TC)],
                                         start=True, stop=True,
                                         tile_position=(64, 0))
                    bcs = npool.tile([64, TH], F32, tag="bcs", name="bcs")
                    nc.vector.tensor_copy(bcs[:], bc[0:64, :])
                    # normalized attn out written straight into the packed
                    # projection tile: head A -> rows 0:64, B -> 64:128
                    # (GPSIMD can't read PSUM, so these stay on DVE)
                    nc.vector.tensor_mul(
                        upk[g][th][64 * (h % 2):64 * (h % 2) + 64, :],
                        u[h][0:64, :], bcs[:])

        # ---- projection: y^T partial [m, t] ----
        for m in range(8):
            for th in range(2):
                for c in range(2):
                    yp = ps_tile([128, TC], stags[(m * 4 + th * 2 + c) % 2])
                    for g in range(2):
                        nc.tensor.matmul(yp[:],
                                         projw[g][:, ts(m, 128)],
                                         upk[g][th][:, ts(c, TC)],
                                         start=(g == 0), stop=(g == 1))
                    yt = ysb.tile([128, TC], F32, tag="yt", name="yt")
                    nc.vector.tensor_copy(yt[:], yp[:])
                    nc.sync.dma_start(
                        y_d[ts(m, 128), th * TH + c * TC:th * TH + (c + 1) * TC],
                        yt[:])


# ---------------- host-side prep / gather ----------------

def rope_tables():
    hd = HD
    inv_freq = 1.0 / (10000.0 ** (np.arange(0, hd, 2, dtype=np.float32) / hd))
    t = np.arange(T, dtype=np.float32)
    freqs = t[:, None] * inv_freq[None, :]                  # [T, 32]
    emb = np.concatenate([np.sin(freqs), np.cos(freqs)], axis=-1)  # [T,64]
    sin_t = emb[:, 0::2].T.astype(np.float32)               # [32, T]
    cos_t = emb[:, 1::2].T.astype(np.float32)
    return sin_t, cos_t


def make_in_maps(x, qkv_w, qkv_b, out_w):
    """Returns list of 8 per-core input dicts."""
    bf = ml_dtypes.bfloat16
    sin_t, cos_t = rope_tables()
    cos4 = np.tile(cos_t, (4, 1)).astype(bf)
    sin4 = np.tile(sin_t, (4, 1)).astype(bf)
    ind = np.zeros((128, 128), np.float32)
    ind[64, 0:64] = 1.0
    ev = np.arange(0, HD, 2)
    od = np.arange(1, HD, 2)

    in_maps = []
    for core in range(8):
        b = core // 4
        h0 = HPC * (core % 4)
        heads = np.arange(h0, h0 + HPC)
        qA = np.concatenate([h * HD + ev for h in heads])          # 128
        qB = np.concatenate([h * HD + od for h in heads])
        kA = DIM + qA
        kB = DIM + qB
        qk_rows = np.concatenate([qA, qB, kA, kB])                  # 512
        v_rows = 2 * DIM + np.arange(h0 * HD, (h0 + HPC) * HD)      # 256
        wqkT = np.ascontiguousarray(qkv_w[qk_rows, :].T).astype(bf)  # [1024,512]
        qkb = np.ascontiguousarray(
            qkv_b[qk_rows].reshape(4, 128).T).astype(np.float32)     # [128,4]
        wvT = np.ascontiguousarray(qkv_w[v_rows, :].T).astype(bf)    # [1024,256]
        vb_bc = np.broadcast_to(qkv_b[v_rows], (128, 256)).astype(np.float32)
        projwT = np.ascontiguousarray(
            out_w[:, h0 * HD:(h0 + HPC) * HD].T).astype(bf)          # [256,1024]
        xT = np.ascontiguousarray(x[b].T).astype(bf)                 # [1024,2048]
        in_maps.append({
            "xT": np.asarray(xT), "wqkT": np.asarray(wqkT), "qkb": qkb,
            "wvT": np.asarray(wvT), "vb_bc": np.ascontiguousarray(vb_bc),
            "cos4": np.asarray(cos4), "sin4": np.asarray(sin4),
            "ind": ind, "projwT": np.asarray(projwT),
        })
    return in_maps


def gather(results, out_b):
    """results: list of 8 dicts with y [1024, 2048] f32 partials."""
    y = np.zeros((B, T, DIM), np.float32)
    for core in range(8):
        b = core // 4
        y[b] += results[core]["y"].T
    y += out_b[None, None, :]
    return y


# ---------------- harness entry point ----------------

_NC_CACHE = {}


def kernel(x, qkv_w, qkv_b, out_w, out_b):
    """Full-input entry: shards across 8 NeuronCores, returns full output."""
    from concourse import bass_utils
    x = np.asarray(x); qkv_w = np.asarray(qkv_w); qkv_b = np.asarray(qkv_b)
    out_w = np.asarray(out_w); out_b = np.asarray(out_b)
    if "nc" not in _NC_CACHE:
        _NC_CACHE["nc"] = build(n_cores=8)
    nc = _NC_CACHE["nc"]
    in_maps = make_in_maps(x, qkv_w, qkv_b, out_w)
    res = bass_utils.run_bass_kernel_spmd(nc, in_maps, core_ids=list(range(8)))
    return gather(res.results, out_b)
